# revision 1
# baseline (speedup 1.0000x reference)
"""Global-attention kernel for [8, 384, 32, 32] ConvAttention on 8 trn2 cores.

Math (per reference): tokens over B*H*W = 8192 positions, C = 384 channels
split as V/K/Q of 128 each; out = softmax(Q K^T / sqrt(128)) V, re-laid as
[B, 128, H, W].

Sharding: core c owns the 1024 query tokens of batch c (token n = b*1024+hw,
so batch == contiguous token block). K/V are replicated. Each core computes
its row block of the attention entirely locally; no collectives.

On-core layout: everything channel-major ([d, token]) which is exactly how
x is laid out in DRAM, so host prep is just slicing + two cheap transposes:
  qT [128, 1024]  = x[c, 256:384].reshape(128, 1024)          (per core)
  kT [128, 8192]  = x[:, 128:256] tokens, channel-major        (replicated)
  vt [128, 8192]  = V tokens chunk-transposed: vt[p, 128*j+v] = V[128*j+p, v]
The S^T = K_chunk Q^T matmul then needs no on-chip transposes at all, the
softmax denominator comes from a ones-vector matmul (partition reduction on
PE), and the output lands directly in [v, token] = DRAM layout.
"""

import math

import numpy as np

import concourse.bass as bass
import concourse.tile as tile
from concourse import bacc, mybir
from concourse.bass_utils import run_bass_kernel_spmd

N_CORES = 8
B, C, H, W = 8, 384, 32, 32
HW = H * W            # 1024 tokens per batch == per core
N = B * HW            # 8192 total tokens
D = 128               # key/value width
NCHUNK = N // 128     # 64 kv chunks of 128 tokens
SCALE = 1.0 / math.sqrt(D)
F32 = mybir.dt.float32
F32R = mybir.dt.float32r

# Rowsum work split: chunks 0..RS_PE_CHUNKS-1 reduce on PE (ones-matmul),
# the rest accumulate elementwise on the otherwise-idle DVE and get folded
# in with one final ones-matmul.
RS_PE_CHUNKS = 64  # v1: all on PE; tune later


def _build_nc():
    nc = bacc.Bacc(
        "TRN2", target_bir_lowering=False, debug=False, num_devices=N_CORES
    )
    qT = nc.dram_tensor("qT", [D, HW], F32, kind="ExternalInput").ap()
    kT = nc.dram_tensor("kT", [D, N], F32, kind="ExternalInput").ap()
    vt = nc.dram_tensor("vt", [D, N], F32, kind="ExternalInput").ap()
    ones = nc.dram_tensor("ones", [D, 1], F32, kind="ExternalInput").ap()
    oT = nc.dram_tensor("oT", [D, HW], F32, kind="ExternalOutput").ap()

    with tile.TileContext(nc) as tc:
        with (
            tc.tile_pool(name="persist", bufs=1) as persist,
            tc.tile_pool(name="etile", bufs=6) as epool,
            tc.tile_pool(name="spsum", bufs=2, space="PSUM") as spsum,
            tc.tile_pool(name="apsum", bufs=1, space="PSUM") as apsum,
        ):
            qT_sb = persist.tile([D, HW], F32R, tag="qT_sb")
            ones_sb = persist.tile([D, 1], F32R, tag="ones_sb")
            kT_sb = [persist.tile([D, HW], F32R, tag=f"kT{i}", name=f"kT_sb{i}") for i in range(8)]
            vt_sb = [persist.tile([D, HW], F32R, tag=f"vt{i}", name=f"vt_sb{i}") for i in range(8)]

            nc.sync.dma_start(out=qT_sb[:], in_=qT[:].bitcast(F32R))
            nc.sync.dma_start(out=ones_sb[:], in_=ones[:].bitcast(F32R))
            # Interleave K/V pieces so PV(c) never waits behind the whole
            # K stream.
            for i in range(8):
                nc.sync.dma_start(out=kT_sb[i][:], in_=kT[:, i * HW : (i + 1) * HW].bitcast(F32R))
                nc.sync.dma_start(out=vt_sb[i][:], in_=vt[:, i * HW : (i + 1) * HW].bitcast(F32R))

            o_psum = apsum.tile([D, HW], F32, tag="o_psum")
            rs_psum = apsum.tile([1, HW], F32, tag="rs_psum")

            rs_acc = persist.tile([D, HW], F32, tag="rs_acc")
            nc.vector.memset(rs_acc[:], 0.0)

            def emit_qk(c):
                blk, off = c // 8, (c % 8) * 128
                s_ps = spsum.tile([D, HW], F32, tag="s_ps", name=f"s_ps{c}")
                for h in range(2):
                    nc.tensor.matmul(
                        s_ps[:, h * 512 : (h + 1) * 512],
                        kT_sb[blk][:, off : off + 128],
                        qT_sb[:, h * 512 : (h + 1) * 512],
                        start=True,
                        stop=True,
                    )
                return s_ps

            # Software-pipelined by one chunk: PE's program order is
            # QK(c+1) -> PV(c), so PE streams QK(c+1) while ACT exps S(c)
            # instead of stalling in-order behind PV(c)'s wait.
            s_tiles = {0: emit_qk(0)}
            first_pe_rs = True
            for c in range(NCHUNK):
                if c + 1 < NCHUNK:
                    s_tiles[c + 1] = emit_qk(c + 1)

                e_sb = epool.tile([D, HW], F32R, tag="e_sb", name=f"e_sb{c}")
                nc.scalar.activation(
                    e_sb[:],
                    s_tiles.pop(c)[:],
                    mybir.ActivationFunctionType.Exp,
                    scale=SCALE,
                )

                blk, off = c // 8, (c % 8) * 128
                for h in range(2):
                    nc.tensor.matmul(
                        o_psum[:, h * 512 : (h + 1) * 512],
                        vt_sb[blk][:, off : off + 128],
                        e_sb[:, h * 512 : (h + 1) * 512],
                        start=(c == 0),
                        stop=(c == NCHUNK - 1),
                    )

                # Rowsum: ~1/5 of chunks reduce on PE (ones-matmul), the rest
                # accumulate elementwise on the otherwise-idle DVE; balanced
                # so ACT's exp stream stays the critical path.
                if c % 5 == 4:
                    for h in range(2):
                        nc.tensor.matmul(
                            rs_psum[:, h * 512 : (h + 1) * 512],
                            ones_sb[:],
                            e_sb[:, h * 512 : (h + 1) * 512],
                            start=first_pe_rs,
                            stop=False,
                        )
                    first_pe_rs = False
                else:
                    nc.vector.tensor_add(
                        rs_acc[:], rs_acc[:], e_sb[:].bitcast(F32)
                    )

            # Fold the DVE partial sums into the PSUM rowsum (via an f32r
            # copy so the fp32r matmul verifier sees a rounded producer).
            rs_acc_r = persist.tile([D, HW], F32R, tag="rs_acc_r")
            nc.scalar.copy(rs_acc_r[:], rs_acc[:])
            for h in range(2):
                nc.tensor.matmul(
                    rs_psum[:, h * 512 : (h + 1) * 512],
                    ones_sb[:],
                    rs_acc_r[:, h * 512 : (h + 1) * 512],
                    start=first_pe_rs,
                    stop=True,
                )

            # softmax denominator -> reciprocal -> scale columns of o_psum
            rs_sb = persist.tile([1, HW], F32, tag="rs_sb")
            nc.scalar.copy(rs_sb[:], rs_psum[:])
            recip_sb = persist.tile([1, HW], F32, tag="recip_sb")
            nc.vector.reciprocal(recip_sb[:], rs_sb[:])
            bc_sb = persist.tile([D, HW], F32, tag="bc_sb")
            nc.gpsimd.partition_broadcast(bc_sb[:], recip_sb[:])
            o_sb = persist.tile([D, HW], F32, tag="o_sb")
            nc.vector.tensor_mul(o_sb[:], o_psum[:], bc_sb[:])
            nc.sync.dma_start(out=oT[:], in_=o_sb[:])

    nc.compile()
    return nc


_NC_CACHE = None


def _get_nc():
    global _NC_CACHE
    if _NC_CACHE is None:
        _NC_CACHE = _build_nc()
    return _NC_CACHE


def kernel(x: np.ndarray) -> np.ndarray:
    assert x.shape == (B, C, H, W), x.shape
    x = np.ascontiguousarray(x, dtype=np.float32)
    xr = x.reshape(B, C, HW)

    # K channel-major over all tokens: kT[d, b*1024+hw] = x[b, 128+d, hw]
    kT = np.ascontiguousarray(xr[:, 128:256, :].transpose(1, 0, 2)).reshape(D, N)
    # V chunk-transposed: vt[p, 128*j + v] = V[128*j + p, v],
    # V[n, v] = x[b, v, hw] with n = b*1024 + hw
    v_tok = np.ascontiguousarray(xr[:, 0:128, :].transpose(0, 2, 1)).reshape(N, D)
    vt = np.ascontiguousarray(v_tok.reshape(NCHUNK, 128, D).transpose(1, 0, 2)).reshape(
        D, N
    )

    ones_col = np.ones((D, 1), dtype=np.float32)
    in_maps = []
    for c in range(N_CORES):
        qT = np.ascontiguousarray(xr[c, 256:384, :])
        in_maps.append({"qT": qT, "kT": kT, "vt": vt, "ones": ones_col})

    nc = _get_nc()
    res = run_bass_kernel_spmd(nc, in_maps, list(range(N_CORES)))

    out = np.empty((B, D, H, W), dtype=np.float32)
    for c in range(N_CORES):
        out[c] = res.results[c]["oT"].reshape(D, H, W)
    return out



# revision 14
# speedup vs baseline: 1.3133x; 1.3133x over previous
"""Global-attention kernel for [8, 384, 32, 32] ConvAttention on 8 trn2 cores.

Math (per reference): tokens over B*H*W = 8192 positions, C = 384 channels
split as V/K/Q of 128 each; out = softmax(Q K^T / sqrt(128)) V, re-laid as
[B, 128, H, W].

Sharding: core c owns the 1024 query tokens of batch c (token n = b*1024+hw,
so batch == contiguous token block). K/V are replicated. Each core computes
its row block of the attention entirely locally; no collectives.

On-core layout: channel-major ([d, token]) everywhere, S^T formulation:
for each kv chunk j (128 tokens), S^T_j = K_j^T Q in PSUM, exp on ACT (or a
Schraudolph bit-trick exp on DVE for a minority of chunks, to split the
elementwise load across both engines), then O^T += V_j^T E_j on PE.

The e tiles are fp16 end-to-end: ACT writes fp16, the DVE trick writes int16
holding the fp16 bit pattern, and the softmax-denominator accumulation runs
as fp16 tensor_adds on DVE (2x perf mode). The denominator fold + partition
broadcast is one ones-matrix matmul; normalize = reciprocal + multiply.
"""

import math

import numpy as np

import concourse.bass as bass
import concourse.tile as tile
from concourse import bacc, mybir
from concourse.alu_op_type import AluOpType
from concourse.bass_utils import run_bass_kernel_spmd

N_CORES = 8
B, C, H, W = 8, 384, 32, 32
HW = H * W            # 1024 tokens per batch == per core
N = B * HW            # 8192 total tokens
D = 128               # key/value width
NCHUNK = N // 128     # 64 kv chunks of 128 tokens
SCALE = 1.0 / math.sqrt(D)
F32 = mybir.dt.float32
F32R = mybir.dt.float32r
F16 = mybir.dt.float16
BF16 = mybir.dt.bfloat16
I16 = mybir.dt.int16

# Raw logits reach ~|21| after scaling, so exp spans ~e^-21..e^21 — far
# beyond fp16 range. All e-side tensors are bf16 (fp32's exponent range).

# Schraudolph exp on DVE (bf16 bit trick): i16 = x*a + b truncated to int16;
# the bit pattern read as bf16 approximates exp(x*SCALE) within ~3.5%.
A7S = float((1 << 7) / math.log(2.0) * SCALE)
B7 = float(127 * (1 << 7) - 6)

# Chunks whose exp runs on DVE via the bit trick (the rest exp on ACT).
DVE_CHUNKS = frozenset(c for c in range(NCHUNK) if c % 6 == 2)  # 11 of 64

N_WARMUP_MM = 14  # PE p-state warmup matmuls issued while input DMAs land


def _build_nc():
    nc = bacc.Bacc(
        "TRN2", target_bir_lowering=False, debug=False, num_devices=N_CORES
    )
    qT = nc.dram_tensor("qT", [D, HW], F32, kind="ExternalInput").ap()
    kT00 = nc.dram_tensor("kT00", [D, D], F32, kind="ExternalInput").ap()
    kT = nc.dram_tensor("kT", [D, N], F32, kind="ExternalInput").ap()
    vt16 = nc.dram_tensor("vt16", [D, N], F16, kind="ExternalInput").ap()
    oT = nc.dram_tensor("oT", [D, HW], F32, kind="ExternalOutput").ap()

    with tile.TileContext(nc) as tc:
        with (
            tc.tile_pool(name="persist", bufs=1) as persist,
            tc.tile_pool(name="etile", bufs=8) as epool,
            tc.tile_pool(name="spsum", bufs=3, space="PSUM") as spsum,
            tc.tile_pool(name="apsum", bufs=1, space="PSUM") as apsum,
        ):
            # --- SBUF persistents ---
            qT_sb = persist.tile([D, HW], F32R, tag="qT_sb")
            kT00_sb = persist.tile([D, D], F32R, tag="kT00_sb")
            kT_sb = [
                persist.tile([D, HW], F32R, tag=f"kT{i}", name=f"kT_sb{i}")
                for i in range(8)
            ]
            vt_sb = [
                persist.tile([D, HW], F16, tag=f"vt{i}", name=f"vt_sb{i}")
                for i in range(8)
            ]
            ones16 = persist.tile([D, D], BF16, tag="ones16")
            rs16 = persist.tile([D, HW], BF16, tag="rs16")
            warm_sb = persist.tile([D, 256], F32, tag="warm_sb")

            # memset only takes fp32 values; convert to bf16 on Pool (idle).
            scr32 = persist.tile([D, HW], F32, tag="scr32")
            nc.gpsimd.memset(scr32[:], 1.0)
            nc.gpsimd.tensor_copy(ones16[:], scr32[:, 0:D])
            nc.gpsimd.memset(scr32[:], 0.0)
            nc.gpsimd.tensor_copy(rs16[:], scr32[:])
            nc.gpsimd.memset(warm_sb[:], 0.5)

            # --- input DMAs, latency-critical pieces first ---
            nc.sync.dma_start(out=kT00_sb[:], in_=kT00[:].bitcast(F32R))
            nc.sync.dma_start(out=qT_sb[:, 0:512], in_=qT[:, 0:512].bitcast(F32R))
            nc.sync.dma_start(out=qT_sb[:, 512:1024], in_=qT[:, 512:1024].bitcast(F32R))
            for i in range(8):
                nc.sync.dma_start(
                    out=kT_sb[i][:], in_=kT[:, i * HW : (i + 1) * HW].bitcast(F32R)
                )
                nc.sync.dma_start(
                    out=vt_sb[i][:], in_=vt16[:, i * HW : (i + 1) * HW]
                )

            # --- PE warmup: keep the tensor engine busy (and ramping to full
            # clock) while the first input DMAs land. Results are discarded.
            wm_ps = spsum.tile([D, HW], F32, tag="s", name="warm_ps")
            for i in range(N_WARMUP_MM):
                nc.tensor.matmul(
                    wm_ps[:, 0:256],
                    warm_sb[:, 0:128].bitcast(F32R),
                    warm_sb[:].bitcast(F32R),
                    start=True,
                    stop=True,
                )

            o_psum = apsum.tile([D, HW], F32, tag="o_psum")

            def kchunk(c):
                if c == 0:
                    return kT00_sb[:]
                blk, off = c // 8, (c % 8) * 128
                return kT_sb[blk][:, off : off + 128]

            def emit_qk(c):
                s_ps = spsum.tile([D, HW], F32, tag="s", name=f"s_ps{c}")
                for h in range(2):
                    nc.tensor.matmul(
                        s_ps[:, h * 512 : (h + 1) * 512],
                        kchunk(c),
                        qT_sb[:, h * 512 : (h + 1) * 512],
                        start=True,
                        stop=True,
                    )
                return s_ps

            # Software-pipelined two chunks ahead (3 PSUM S-slots).
            s_tiles = {0: emit_qk(0), 1: emit_qk(1)}
            for c in range(NCHUNK):
                if c + 2 < NCHUNK:
                    s_tiles[c + 2] = emit_qk(c + 2)
                s_ps = s_tiles.pop(c)

                if c in DVE_CHUNKS:
                    e_i16 = epool.tile([D, HW], I16, tag="e", name=f"e{c}")
                    nc.vector.tensor_scalar(
                        out=e_i16[:],
                        in0=s_ps[:],
                        scalar1=A7S,
                        scalar2=B7,
                        op0=AluOpType.mult,
                        op1=AluOpType.add,
                    )
                    e16 = e_i16[:].bitcast(BF16)
                else:
                    e_sb = epool.tile([D, HW], BF16, tag="e", name=f"e{c}")
                    nc.scalar.activation(
                        e_sb[:],
                        s_ps[:],
                        mybir.ActivationFunctionType.Exp,
                        scale=SCALE,
                    )
                    e16 = e_sb[:]

                blk, off = c // 8, (c % 8) * 128
                for h in range(2):
                    nc.tensor.matmul(
                        o_psum[:, h * 512 : (h + 1) * 512],
                        vt_sb[blk][:, off : off + 128],
                        e16[:, h * 512 : (h + 1) * 512],
                        start=(c == 0),
                        stop=(c == NCHUNK - 1),
                    )

                # softmax denominator partials: 2-byte adds run in DVE 2x mode
                nc.vector.tensor_add(rs16[:], rs16[:], e16)

            # --- endgame: fold+broadcast denominator, normalize, store ---
            # ones16 @ rs16 gives, in every output partition, the full
            # partition-sum of rs16 -> denominator replicated 128x.
            rs_bc_ps = spsum.tile([D, HW], F32, tag="s", name="rs_bc_ps")
            for h in range(2):
                nc.tensor.matmul(
                    rs_bc_ps[:, h * 512 : (h + 1) * 512],
                    ones16[:],
                    rs16[:, h * 512 : (h + 1) * 512],
                    start=True,
                    stop=True,
                )
            for h in range(2):
                sl = slice(h * 512, (h + 1) * 512)
                rec_sb = persist.tile([D, 512], F32, tag=f"rec{h}")
                nc.vector.reciprocal(rec_sb[:], rs_bc_ps[:, sl])
                o_sb = persist.tile([D, 512], F32, tag=f"osb{h}")
                nc.vector.tensor_tensor(
                    o_sb[:], o_psum[:, sl], rec_sb[:], AluOpType.mult
                )
                nc.sync.dma_start(out=oT[:, sl], in_=o_sb[:])

    nc.compile()
    return nc


_NC_CACHE = None


def _get_nc():
    global _NC_CACHE
    if _NC_CACHE is None:
        _NC_CACHE = _build_nc()
    return _NC_CACHE


def _prep_inputs(x: np.ndarray) -> list[dict]:
    x = np.ascontiguousarray(x, dtype=np.float32)
    xr = x.reshape(B, C, HW)

    # K channel-major over all tokens: kT[d, b*1024+hw] = x[b, 128+d, hw]
    kT = np.ascontiguousarray(xr[:, 128:256, :].transpose(1, 0, 2)).reshape(D, N)
    kT00 = np.ascontiguousarray(kT[:, 0:128])
    # V chunk-transposed fp16: vt[p, 128*j + v] = V[128*j + p, v]
    v_tok = np.ascontiguousarray(xr[:, 0:128, :].transpose(0, 2, 1)).reshape(N, D)
    vt16 = np.ascontiguousarray(
        v_tok.reshape(NCHUNK, 128, D).transpose(1, 0, 2)
    ).reshape(D, N).astype(np.float16)

    in_maps = []
    for c in range(N_CORES):
        qT = np.ascontiguousarray(xr[c, 256:384, :])
        in_maps.append({"qT": qT, "kT00": kT00, "kT": kT, "vt16": vt16})
    return in_maps


def kernel(x: np.ndarray) -> np.ndarray:
    assert x.shape == (B, C, H, W), x.shape
    in_maps = _prep_inputs(x)
    nc = _get_nc()
    res = run_bass_kernel_spmd(nc, in_maps, list(range(N_CORES)))

    out = np.empty((B, D, H, W), dtype=np.float32)
    for c in range(N_CORES):
        out[c] = res.results[c]["oT"].reshape(D, H, W)
    return out
